# revision 15
# baseline (speedup 1.0000x reference)
"""MoE router kernel for Trainium2 (8 NeuronCores, SPMD data-parallel).

logits = x @ W.T ; probs = softmax(logits) ; top-2 + renormalized weights.

Sharding: x (4,8192,4096) -> (32768,4096) tokens, 4096 tokens per core;
W (64,4096) replicated.

Per-core pipeline (all fp32):
  1. DMA x block natural [128t, 1024d] tiles.
  2. PE transpose-mode flips 128x128 tiles -> PSUM (x.T layout).
  3. DVE/ACT drain PSUM -> SBUF [128d, 512t] fp32 tiles.
  4. PE matmuls, col-tiled 2x (two 512-token halves concurrently in
     col-groups 0-1 / 2-3): psum[0:64]=W.T_c.T @ xT_A, psum[64:128]=...B,
     accumulated over 32 d-chunks -> logits.T [2x64e, 512t].
  5. PE transposes logits back to [128t, 64e]; softmax on DVE/ACT
     (exp with fused row-sum accum), top-2 via DVE max/max_index.
  6. DMA probs/indices/weights out.
"""
import sys

sys.path.insert(0, "/opt/trn_rl_repo")

import numpy as np

N_CORES = 8
T_CORE = 4096  # tokens per core
D = 4096
E = 64
TB = 1024  # token block
DS = 1024  # d slice
N_BLOCKS = T_CORE // TB
N_SLICES = D // DS


def _split_waits(nc, max_waits=1):
    """walrus rejects instructions with >N sem waits; push overflow waits
    onto NOPs inserted before, on the same engine."""
    from concourse import mybir

    for f in nc.m.functions:
        for blk in f.blocks:
            insts = list(blk.instructions)
            out = []
            changed = False
            for inst in insts:
                si = inst.sync_info
                waits = list(si.on_wait) if si is not None else []
                if len(waits) > max_waits:
                    overflow = waits[:-max_waits]
                    keep = waits[-max_waits:]
                    for i in range(0, len(overflow), max_waits):
                        chunk = overflow[i : i + max_waits]
                        nop = mybir.InstNoOp(
                            name=nc.get_next_instruction_name(),
                            engine=inst.engine,
                            ins=[],
                            outs=[],
                            sync_info=mybir.SyncInfo(on_wait=chunk, on_update=[]),
                        )
                        nc.register_instruction(nop)
                        out.append(nop)
                    inst.sync_info = mybir.SyncInfo(
                        on_wait=keep, on_update=list(si.on_update)
                    )
                    changed = True
                out.append(inst)
            if changed:
                blk.instructions = out


def build_nc():
    import concourse.bass as bass
    import concourse.mybir as mybir
    from concourse import tile

    f32 = mybir.dt.float32

    nc = bass.Bass()
    x_d = nc.dram_tensor("x", [T_CORE, D], f32, kind="ExternalInput")
    w_d = nc.dram_tensor("w", [E, D], f32, kind="ExternalInput")
    i128_d = nc.dram_tensor("i128", [128, 128], f32, kind="ExternalInput")
    i64_d = nc.dram_tensor("i64", [128, 64], f32, kind="ExternalInput")
    probs_d = nc.dram_tensor("probs", [T_CORE, E], f32, kind="ExternalOutput")
    idx_d = nc.dram_tensor("idx", [T_CORE, 2], mybir.dt.int32, kind="ExternalOutput")
    wts_d = nc.dram_tensor("wts", [T_CORE, 2], f32, kind="ExternalOutput")

    with tile.TileContext(nc) as tc:
        with (
            tc.tile_pool(name="const", bufs=1) as constp,
            tc.tile_pool(name="xn", bufs=24) as xnp,
            tc.tile_pool(name="xt", bufs=4) as xtp,
            tc.tile_pool(name="sm", bufs=8) as smp,
            tc.tile_pool(name="outp", bufs=2) as outp,
            tc.tile_pool(name="pslg", bufs=1, space="PSUM") as pslg,
            tc.tile_pool(name="pslg2", bufs=1, space="PSUM") as pslg2,
            tc.tile_pool(name="pstr", bufs=3, space="PSUM") as pstr,
            tc.tile_pool(name="psptr", bufs=2, space="PSUM") as psptr,
        ):
            def load_slice(b, s_):
                tiles = []
                for j in range(8):
                    xn = xnp.tile([128, DS], f32, tag="xn")
                    nc.sync.dma_start(
                        xn[:],
                        x_d[
                            b * TB + j * 128 : b * TB + (j + 1) * 128,
                            s_ * DS : (s_ + 1) * DS,
                        ],
                    )
                    tiles.append(xn)
                return tiles

            i128 = constp.tile([128, 128], f32)
            nc.sync.dma_start(i128[:], i128_d[:, :])
            i64 = constp.tile([128, 64], f32)
            nc.sync.dma_start(i64[:], i64_d[:, :])
            w_sb = constp.tile([64, D], f32)
            nc.sync.dma_start(w_sb[:], w_d[:, :])

            first_tiles = load_slice(0, 0)

            # ---- W.T setup: wt[:, 64c:64c+64] = W[:, 128c:128c+128].T ----
            wt = constp.tile([128, 32 * E], f32)  # 32 chunks of [128d, 64e]
            for g in range(4):
                pw = pstr.tile([128, 512], f32, tag="tr")
                for q in range(8):
                    c = g * 8 + q
                    nc.tensor.transpose(
                        pw[:, 64 * q : 64 * (q + 1)],
                        w_sb[:, 128 * c : 128 * (c + 1)],
                        i64[0:64, :],
                    )
                nc.vector.tensor_copy(wt[:, 512 * g : 512 * (g + 1)], pw[:])

            # ---- main loop ----
            for b in range(N_BLOCKS):
                lgA = pslg.tile([128, 512], f32, tag="lgA")
                lgB = pslg2.tile([128, 512], f32, tag="lgB")
                for s in range(N_SLICES):
                    if b == 0 and s == 0:
                        xts = first_tiles
                    else:
                        xts = load_slice(b, s)
                    for c in range(8):
                        kchunk = s * 8 + c
                        first = kchunk == 0
                        last = kchunk == 31
                        xt_sb = xtp.tile([128, 1024], f32, tag="xt")
                        for h in range(2):
                            tr = pstr.tile([128, 512], f32, tag="tr")
                            for q in range(4):
                                nc.tensor.transpose(
                                    tr[:, 128 * q : 128 * (q + 1)],
                                    xts[h * 4 + q][:, 128 * c : 128 * (c + 1)],
                                    i128[:],
                                )
                            if h == 0:
                                nc.vector.tensor_copy(xt_sb[:, 0:512], tr[:])
                            else:
                                nc.scalar.copy(xt_sb[:, 512:1024], tr[:])
                        xT = [xt_sb[:, 0:512], xt_sb[:, 512:1024]]
                        wchunk = wt[:, 64 * kchunk : 64 * (kchunk + 1)]
                        boost = 48 if (b > 0 or kchunk >= 2) else 0
                        with tc.high_priority(offset=boost):
                            nc.tensor.matmul(
                                lgA[0:64, :], wchunk, xT[0],
                                start=first, stop=last, tile_position=(0, 0),
                            )
                            nc.tensor.matmul(
                                lgB[64:128, :], wchunk, xT[1],
                                start=first, stop=last, tile_position=(0, 64),
                            )

                # logits.T for block b: lg = [2 halves x 64e, 512t]
                lgs = outp.tile([128, 512], f32, tag="lgs")
                nc.vector.tensor_copy(lgs[0:64, :], lgA[0:64, :])
                nc.scalar.copy(lgs[64:128, :], lgB[64:128, :])
                ptr = psptr.tile([128, 512], f32, tag="ptr")
                for h in range(2):
                    for q in range(4):
                        k = h * 4 + q
                        nc.tensor.transpose(
                            ptr[:, 64 * k : 64 * (k + 1)],
                            lgs[64 * h : 64 * (h + 1), 128 * q : 128 * (q + 1)],
                            i64[64 * h : 64 * (h + 1), :],
                        )
                probs_sb = outp.tile([128, 512], f32, tag="probs")
                idx_sb = outp.tile([128, 16], mybir.dt.uint32, tag="idx")
                wts_sb = outp.tile([128, 16], f32, tag="wts")
                for k in range(8):
                    pt = ptr[:, 64 * k : 64 * (k + 1)]
                    pslice = probs_sb[:, 64 * k : 64 * (k + 1)]
                    mneg = smp.tile([128, 1], f32, tag="mneg")
                    nc.vector.tensor_reduce(
                        mneg[:], pt, axis=mybir.AxisListType.X,
                        op=mybir.AluOpType.max, negate=True,
                    )
                    ssum = smp.tile([128, 1], f32, tag="ssum")
                    nc.scalar.activation(
                        pslice, pt, mybir.ActivationFunctionType.Exp,
                        bias=mneg[:], scale=1.0, accum_out=ssum[:],
                    )
                    rr = smp.tile([128, 1], f32, tag="rr")
                    nc.vector.reciprocal(rr[:], ssum[:])
                    nc.scalar.activation(
                        pslice, pslice, mybir.ActivationFunctionType.Copy,
                        scale=rr[:],
                    )
                    mx = smp.tile([128, 8], f32, tag="mx")
                    nc.vector.max(mx[:], pslice)
                    ix = smp.tile([128, 8], mybir.dt.uint32, tag="ix")
                    nc.vector.max_index(ix[:], mx[:], pslice)
                    nc.vector.tensor_copy(idx_sb[:, 2 * k : 2 * k + 2], ix[:, 0:2])
                    s2 = smp.tile([128, 1], f32, tag="s2")
                    nc.vector.tensor_reduce(
                        s2[:], mx[:, 0:2], axis=mybir.AxisListType.X,
                        op=mybir.AluOpType.add,
                    )
                    r2 = smp.tile([128, 1], f32, tag="r2")
                    nc.vector.reciprocal(r2[:], s2[:])
                    nc.vector.tensor_scalar_mul(
                        wts_sb[:, 2 * k : 2 * k + 2], mx[:, 0:2], r2[:]
                    )
                # outputs for block b
                nc.scalar.dma_start(
                    probs_d[b * TB : (b + 1) * TB, :].rearrange(
                        "(k p) e -> p k e", p=128
                    ),
                    probs_sb[:],
                )
                nc.scalar.dma_start(
                    idx_d[b * TB : (b + 1) * TB, :].rearrange(
                        "(k p) e -> p k e", p=128
                    ),
                    idx_sb[:].bitcast(mybir.dt.int32),
                )
                nc.scalar.dma_start(
                    wts_d[b * TB : (b + 1) * TB, :].rearrange(
                        "(k p) e -> p k e", p=128
                    ),
                    wts_sb[:],
                )

    _split_waits(nc, max_waits=1)
    return nc


_NC_CACHE = {}


def _get_nc():
    if "nc" not in _NC_CACHE:
        _NC_CACHE["nc"] = build_nc()
    return _NC_CACHE["nc"]


def kernel(x, W):
    from concourse.bass_utils import run_bass_kernel_spmd

    x = np.asarray(x, dtype=np.float32)
    W = np.asarray(W, dtype=np.float32)
    B, S, _ = x.shape  # (4, 8192, 4096)
    xt = np.ascontiguousarray(x.reshape(B * S, D))
    i128 = np.eye(128, dtype=np.float32)
    i64 = np.concatenate([np.eye(64, dtype=np.float32)] * 2, axis=0)
    in_maps = [
        {
            "x": np.ascontiguousarray(xt[T_CORE * i : T_CORE * (i + 1)]),
            "w": W,
            "i128": i128,
            "i64": i64,
        }
        for i in range(N_CORES)
    ]
    nc = _get_nc()
    r = run_bass_kernel_spmd(nc, in_maps, core_ids=list(range(N_CORES)))
    probs = np.concatenate([r.results[i]["probs"] for i in range(N_CORES)], axis=0)
    idx = np.concatenate([r.results[i]["idx"] for i in range(N_CORES)], axis=0)
    wts = np.concatenate([r.results[i]["wts"] for i in range(N_CORES)], axis=0)
    return (
        probs.reshape(B, S, E),
        idx.reshape(B, S, 2).astype(np.int32),
        wts.reshape(B, S, 2),
    )


# revision 17
# speedup vs baseline: 1.1493x; 1.1493x over previous
"""MoE router kernel for Trainium2 (8 NeuronCores, SPMD data-parallel).

logits = x @ W.T ; probs = softmax(logits) ; top-2 + renormalized weights.

Sharding: x (4,8192,4096) -> (32768,4096) tokens, 4096 tokens per core;
W (64,4096) replicated.

Per-core pipeline (all fp32):
  1. DMA x block natural [128t, 1024d] tiles.
  2. PE transpose-mode flips 128x128 tiles -> PSUM (x.T layout).
  3. DVE/ACT drain PSUM -> SBUF [128d, 512t] fp32 tiles.
  4. PE matmuls, col-tiled 2x (two 512-token halves concurrently in
     col-groups 0-1 / 2-3): psum[0:64]=W.T_c.T @ xT_A, psum[64:128]=...B,
     accumulated over 32 d-chunks -> logits.T [2x64e, 512t].
  5. PE transposes logits back to [128t, 64e]; softmax on DVE/ACT
     (exp with fused row-sum accum), top-2 via DVE max/max_index.
  6. DMA probs/indices/weights out.
"""
import sys

sys.path.insert(0, "/opt/trn_rl_repo")

import numpy as np

N_CORES = 8
T_CORE = 4096  # tokens per core
D = 4096
E = 64
TB = 1024  # token block
DS = 1024  # d slice
N_BLOCKS = T_CORE // TB
N_SLICES = D // DS


def _split_waits(nc, max_waits=1):
    """walrus rejects instructions with >N sem waits; push overflow waits
    onto NOPs inserted before, on the same engine."""
    from concourse import mybir

    for f in nc.m.functions:
        for blk in f.blocks:
            insts = list(blk.instructions)
            out = []
            changed = False
            for inst in insts:
                si = inst.sync_info
                waits = list(si.on_wait) if si is not None else []
                if len(waits) > max_waits:
                    overflow = waits[:-max_waits]
                    keep = waits[-max_waits:]
                    for i in range(0, len(overflow), max_waits):
                        chunk = overflow[i : i + max_waits]
                        nop = mybir.InstNoOp(
                            name=nc.get_next_instruction_name(),
                            engine=inst.engine,
                            ins=[],
                            outs=[],
                            sync_info=mybir.SyncInfo(on_wait=chunk, on_update=[]),
                        )
                        nc.register_instruction(nop)
                        out.append(nop)
                    inst.sync_info = mybir.SyncInfo(
                        on_wait=keep, on_update=list(si.on_update)
                    )
                    changed = True
                out.append(inst)
            if changed:
                blk.instructions = out


def build_nc():
    import concourse.bass as bass
    import concourse.mybir as mybir
    from concourse import tile

    f32 = mybir.dt.float32

    nc = bass.Bass()
    x_d = nc.dram_tensor("x", [T_CORE, D], f32, kind="ExternalInput")
    w_d = nc.dram_tensor("w", [E, D], f32, kind="ExternalInput")
    i128_d = nc.dram_tensor("i128", [128, 128], f32, kind="ExternalInput")
    i64_d = nc.dram_tensor("i64", [128, 64], f32, kind="ExternalInput")
    probs_d = nc.dram_tensor("probs", [T_CORE, E], f32, kind="ExternalOutput")
    idx_d = nc.dram_tensor("idx", [T_CORE, 2], mybir.dt.int32, kind="ExternalOutput")
    wts_d = nc.dram_tensor("wts", [T_CORE, 2], f32, kind="ExternalOutput")

    with tile.TileContext(nc) as tc:
        with (
            tc.tile_pool(name="const", bufs=1) as constp,
            tc.tile_pool(name="xn", bufs=24) as xnp,
            tc.tile_pool(name="xt", bufs=4) as xtp,
            tc.tile_pool(name="sm", bufs=8) as smp,
            tc.tile_pool(name="outp", bufs=2) as outp,
            tc.tile_pool(name="pslg", bufs=1, space="PSUM") as pslg,
            tc.tile_pool(name="pslg2", bufs=1, space="PSUM") as pslg2,
            tc.tile_pool(name="pstr", bufs=3, space="PSUM") as pstr,
            tc.tile_pool(name="psptr", bufs=2, space="PSUM") as psptr,
        ):
            def load_slice(b, s_):
                tiles = []
                for j in range(8):
                    xn = xnp.tile([128, DS], f32, tag="xn")
                    nc.sync.dma_start(
                        xn[:],
                        x_d[
                            b * TB + j * 128 : b * TB + (j + 1) * 128,
                            s_ * DS : (s_ + 1) * DS,
                        ],
                    )
                    tiles.append(xn)
                return tiles

            i128 = constp.tile([128, 128], f32)
            nc.sync.dma_start(i128[:], i128_d[:, :])
            i64 = constp.tile([128, 64], f32)
            nc.sync.dma_start(i64[:], i64_d[:, :])
            first_tiles = load_slice(0, 0)

            w_sb = constp.tile([64, D], f32)
            nc.sync.dma_start(w_sb[:], w_d[:, :])

            # ---- W.T setup: wt[:, 64c:64c+64] = W[:, 128c:128c+128].T ----
            wt = constp.tile([128, 32 * E], f32)  # 32 chunks of [128d, 64e]
            for g in range(4):
                pw = pstr.tile([128, 512], f32, tag="tr")
                for q in range(8):
                    c = g * 8 + q
                    nc.tensor.transpose(
                        pw[:, 64 * q : 64 * (q + 1)],
                        w_sb[:, 128 * c : 128 * (c + 1)],
                        i64[0:64, :],
                    )
                nc.vector.tensor_copy(wt[:, 512 * g : 512 * (g + 1)], pw[:])

            # ---- main loop ----
            for b in range(N_BLOCKS):
                lgA = pslg.tile([128, 512], f32, tag="lgA")
                lgB = pslg2.tile([128, 512], f32, tag="lgB")
                for s in range(N_SLICES):
                    if b == 0 and s == 0:
                        xts = first_tiles
                    else:
                        xts = load_slice(b, s)
                    for c in range(8):
                        kchunk = s * 8 + c
                        first = kchunk == 0
                        last = kchunk == 31
                        xt_sb = xtp.tile([128, 1024], f32, tag="xt")
                        for h in range(2):
                            tr = pstr.tile([128, 512], f32, tag="tr")
                            for q in range(4):
                                nc.tensor.transpose(
                                    tr[:, 128 * q : 128 * (q + 1)],
                                    xts[h * 4 + q][:, 128 * c : 128 * (c + 1)],
                                    i128[:],
                                )
                            if h == 0:
                                nc.vector.tensor_copy(xt_sb[:, 0:512], tr[:])
                            else:
                                nc.scalar.copy(xt_sb[:, 512:1024], tr[:])
                        xT = [xt_sb[:, 0:512], xt_sb[:, 512:1024]]
                        wchunk = wt[:, 64 * kchunk : 64 * (kchunk + 1)]
                        boost = 32 if (b > 0 or kchunk >= 2) else 0
                        with tc.high_priority(offset=boost):
                            nc.tensor.matmul(
                                lgA[0:64, :], wchunk, xT[0],
                                start=first, stop=last, tile_position=(0, 0),
                            )
                            nc.tensor.matmul(
                                lgB[64:128, :], wchunk, xT[1],
                                start=first, stop=last, tile_position=(0, 64),
                            )

                # logits.T for block b: lg = [2 halves x 64e, 512t]
                lgs = outp.tile([128, 512], f32, tag="lgs")
                nc.vector.tensor_copy(lgs[0:64, :], lgA[0:64, :])
                nc.scalar.copy(lgs[64:128, :], lgB[64:128, :])
                ptr = psptr.tile([128, 512], f32, tag="ptr")
                for h in range(2):
                    for q in range(4):
                        k = h * 4 + q
                        nc.tensor.transpose(
                            ptr[:, 64 * k : 64 * (k + 1)],
                            lgs[64 * h : 64 * (h + 1), 128 * q : 128 * (q + 1)],
                            i64[64 * h : 64 * (h + 1), :],
                        )
                probs_sb = outp.tile([128, 512], f32, tag="probs")
                idx_sb = outp.tile([128, 16], mybir.dt.uint32, tag="idx")
                wts_sb = outp.tile([128, 16], f32, tag="wts")
                for k in range(8):
                    pt = ptr[:, 64 * k : 64 * (k + 1)]
                    pslice = probs_sb[:, 64 * k : 64 * (k + 1)]
                    mneg = smp.tile([128, 1], f32, tag="mneg")
                    nc.vector.tensor_reduce(
                        mneg[:], pt, axis=mybir.AxisListType.X,
                        op=mybir.AluOpType.max, negate=True,
                    )
                    ssum = smp.tile([128, 1], f32, tag="ssum")
                    nc.scalar.activation(
                        pslice, pt, mybir.ActivationFunctionType.Exp,
                        bias=mneg[:], scale=1.0, accum_out=ssum[:],
                    )
                    rr = smp.tile([128, 1], f32, tag="rr")
                    nc.vector.reciprocal(rr[:], ssum[:])
                    nc.scalar.activation(
                        pslice, pslice, mybir.ActivationFunctionType.Copy,
                        scale=rr[:],
                    )
                    mx = smp.tile([128, 8], f32, tag="mx")
                    nc.vector.max(mx[:], pslice)
                    ix = smp.tile([128, 8], mybir.dt.uint32, tag="ix")
                    nc.vector.max_index(ix[:], mx[:], pslice)
                    nc.vector.tensor_copy(idx_sb[:, 2 * k : 2 * k + 2], ix[:, 0:2])
                    s2 = smp.tile([128, 1], f32, tag="s2")
                    nc.vector.tensor_reduce(
                        s2[:], mx[:, 0:2], axis=mybir.AxisListType.X,
                        op=mybir.AluOpType.add,
                    )
                    r2 = smp.tile([128, 1], f32, tag="r2")
                    nc.vector.reciprocal(r2[:], s2[:])
                    nc.vector.tensor_scalar_mul(
                        wts_sb[:, 2 * k : 2 * k + 2], mx[:, 0:2], r2[:]
                    )
                # outputs for block b
                nc.scalar.dma_start(
                    probs_d[b * TB : (b + 1) * TB, :].rearrange(
                        "(k p) e -> p k e", p=128
                    ),
                    probs_sb[:],
                )
                nc.scalar.dma_start(
                    idx_d[b * TB : (b + 1) * TB, :].rearrange(
                        "(k p) e -> p k e", p=128
                    ),
                    idx_sb[:].bitcast(mybir.dt.int32),
                )
                nc.scalar.dma_start(
                    wts_d[b * TB : (b + 1) * TB, :].rearrange(
                        "(k p) e -> p k e", p=128
                    ),
                    wts_sb[:],
                )

    _split_waits(nc, max_waits=1)
    return nc


_NC_CACHE = {}


def _get_nc():
    if "nc" not in _NC_CACHE:
        _NC_CACHE["nc"] = build_nc()
    return _NC_CACHE["nc"]


def kernel(x, W):
    from concourse.bass_utils import run_bass_kernel_spmd

    x = np.asarray(x, dtype=np.float32)
    W = np.asarray(W, dtype=np.float32)
    B, S, _ = x.shape  # (4, 8192, 4096)
    xt = np.ascontiguousarray(x.reshape(B * S, D))
    i128 = np.eye(128, dtype=np.float32)
    i64 = np.concatenate([np.eye(64, dtype=np.float32)] * 2, axis=0)
    in_maps = [
        {
            "x": np.ascontiguousarray(xt[T_CORE * i : T_CORE * (i + 1)]),
            "w": W,
            "i128": i128,
            "i64": i64,
        }
        for i in range(N_CORES)
    ]
    nc = _get_nc()
    r = run_bass_kernel_spmd(nc, in_maps, core_ids=list(range(N_CORES)))
    probs = np.concatenate([r.results[i]["probs"] for i in range(N_CORES)], axis=0)
    idx = np.concatenate([r.results[i]["idx"] for i in range(N_CORES)], axis=0)
    wts = np.concatenate([r.results[i]["wts"] for i in range(N_CORES)], axis=0)
    return (
        probs.reshape(B, S, E),
        idx.reshape(B, S, 2).astype(np.int32),
        wts.reshape(B, S, 2),
    )


# revision 18
# speedup vs baseline: 1.1944x; 1.0393x over previous
"""MoE router kernel for Trainium2 (8 NeuronCores, SPMD data-parallel).

logits = x @ W.T ; probs = softmax(logits) ; top-2 + renormalized weights.

Sharding: x (4,8192,4096) -> (32768,4096) tokens, 4096 tokens per core;
W (64,4096) replicated.

Per-core pipeline (all fp32):
  1. DMA x block natural [128t, 1024d] tiles.
  2. PE transpose-mode flips 128x128 tiles -> PSUM (x.T layout).
  3. DVE/ACT drain PSUM -> SBUF [128d, 512t] fp32 tiles.
  4. PE matmuls, col-tiled 2x (two 512-token halves concurrently in
     col-groups 0-1 / 2-3): psum[0:64]=W.T_c.T @ xT_A, psum[64:128]=...B,
     accumulated over 32 d-chunks -> logits.T [2x64e, 512t].
  5. PE transposes logits back to [128t, 64e]; softmax on DVE/ACT
     (exp with fused row-sum accum), top-2 via DVE max/max_index.
  6. DMA probs/indices/weights out.
"""
import sys

sys.path.insert(0, "/opt/trn_rl_repo")

import numpy as np

N_CORES = 8
T_CORE = 4096  # tokens per core
D = 4096
E = 64
TB = 1024  # token block
DS = 1024  # d slice
N_BLOCKS = T_CORE // TB
N_SLICES = D // DS


def _split_waits(nc, max_waits=1):
    """walrus rejects instructions with >N sem waits; push overflow waits
    onto NOPs inserted before, on the same engine."""
    from concourse import mybir

    for f in nc.m.functions:
        for blk in f.blocks:
            insts = list(blk.instructions)
            out = []
            changed = False
            for inst in insts:
                si = inst.sync_info
                waits = list(si.on_wait) if si is not None else []
                if len(waits) > max_waits:
                    overflow = waits[:-max_waits]
                    keep = waits[-max_waits:]
                    for i in range(0, len(overflow), max_waits):
                        chunk = overflow[i : i + max_waits]
                        nop = mybir.InstNoOp(
                            name=nc.get_next_instruction_name(),
                            engine=inst.engine,
                            ins=[],
                            outs=[],
                            sync_info=mybir.SyncInfo(on_wait=chunk, on_update=[]),
                        )
                        nc.register_instruction(nop)
                        out.append(nop)
                    inst.sync_info = mybir.SyncInfo(
                        on_wait=keep, on_update=list(si.on_update)
                    )
                    changed = True
                out.append(inst)
            if changed:
                blk.instructions = out


def build_nc():
    import concourse.bass as bass
    import concourse.mybir as mybir
    from concourse import tile

    f32 = mybir.dt.float32

    nc = bass.Bass()
    x_d = nc.dram_tensor("x", [T_CORE, D], f32, kind="ExternalInput")
    w_d = nc.dram_tensor("w", [E, D], f32, kind="ExternalInput")
    i128_d = nc.dram_tensor("i128", [128, 128], f32, kind="ExternalInput")
    i64_d = nc.dram_tensor("i64", [128, 64], f32, kind="ExternalInput")
    probs_d = nc.dram_tensor("probs", [T_CORE, E], f32, kind="ExternalOutput")
    idx_d = nc.dram_tensor("idx", [T_CORE, 2], mybir.dt.int32, kind="ExternalOutput")
    wts_d = nc.dram_tensor("wts", [T_CORE, 2], f32, kind="ExternalOutput")

    with tile.TileContext(nc) as tc:
        with (
            tc.tile_pool(name="const", bufs=1) as constp,
            tc.tile_pool(name="xn", bufs=24) as xnp,
            tc.tile_pool(name="xt", bufs=6) as xtp,
            tc.tile_pool(name="sm", bufs=8) as smp,
            tc.tile_pool(name="outp", bufs=2) as outp,
            tc.tile_pool(name="pslg", bufs=1, space="PSUM") as pslg,
            tc.tile_pool(name="pslg2", bufs=1, space="PSUM") as pslg2,
            tc.tile_pool(name="pstr", bufs=3, space="PSUM") as pstr,
            tc.tile_pool(name="psptr", bufs=2, space="PSUM") as psptr,
        ):
            def load_slice(b, s_):
                tiles = []
                for j in range(8):
                    xn = xnp.tile([128, DS], f32, tag="xn")
                    nc.sync.dma_start(
                        xn[:],
                        x_d[
                            b * TB + j * 128 : b * TB + (j + 1) * 128,
                            s_ * DS : (s_ + 1) * DS,
                        ],
                    )
                    tiles.append(xn)
                return tiles

            i128 = constp.tile([128, 128], f32)
            nc.sync.dma_start(i128[:], i128_d[:, :])
            i64 = constp.tile([128, 64], f32)
            nc.sync.dma_start(i64[:], i64_d[:, :])
            w_sb = constp.tile([64, D], f32)
            nc.sync.dma_start(w_sb[:], w_d[:, :])

            first_tiles = load_slice(0, 0)

            # ---- W.T setup: wt[:, 64c:64c+64] = W[:, 128c:128c+128].T ----
            wt = constp.tile([128, 32 * E], f32)  # 32 chunks of [128d, 64e]
            for g in range(4):
                pw = pstr.tile([128, 512], f32, tag="tr")
                for q in range(8):
                    c = g * 8 + q
                    nc.tensor.transpose(
                        pw[:, 64 * q : 64 * (q + 1)],
                        w_sb[:, 128 * c : 128 * (c + 1)],
                        i64[0:64, :],
                    )
                nc.vector.tensor_copy(wt[:, 512 * g : 512 * (g + 1)], pw[:])

            # ---- main loop ----
            for b in range(N_BLOCKS):
                lgA = pslg.tile([128, 512], f32, tag="lgA")
                lgB = pslg2.tile([128, 512], f32, tag="lgB")
                for s in range(N_SLICES):
                    if b == 0 and s == 0:
                        xts = first_tiles
                    else:
                        xts = load_slice(b, s)
                    for c in range(8):
                        kchunk = s * 8 + c
                        first = kchunk == 0
                        last = kchunk == 31
                        xt_sb = xtp.tile([128, 1024], f32, tag="xt")
                        for h in range(2):
                            tr = pstr.tile([128, 512], f32, tag="tr")
                            for q in range(4):
                                nc.tensor.transpose(
                                    tr[:, 128 * q : 128 * (q + 1)],
                                    xts[h * 4 + q][:, 128 * c : 128 * (c + 1)],
                                    i128[:],
                                )
                            if h == 0:
                                nc.vector.tensor_copy(xt_sb[:, 0:512], tr[:])
                            else:
                                nc.scalar.copy(xt_sb[:, 512:1024], tr[:])
                        xT = [xt_sb[:, 0:512], xt_sb[:, 512:1024]]
                        wchunk = wt[:, 64 * kchunk : 64 * (kchunk + 1)]
                        boost = 32 if (b > 0 or kchunk >= 2) else 0
                        with tc.high_priority(offset=boost):
                            nc.tensor.matmul(
                                lgA[0:64, :], wchunk, xT[0],
                                start=first, stop=last, tile_position=(0, 0),
                            )
                            nc.tensor.matmul(
                                lgB[64:128, :], wchunk, xT[1],
                                start=first, stop=last, tile_position=(0, 64),
                            )

                # logits.T for block b: lg = [2 halves x 64e, 512t]
                lgs = outp.tile([128, 512], f32, tag="lgs")
                nc.vector.tensor_copy(lgs[0:64, :], lgA[0:64, :])
                nc.scalar.copy(lgs[64:128, :], lgB[64:128, :])
                ptr = psptr.tile([128, 512], f32, tag="ptr")
                for h in range(2):
                    for q in range(4):
                        k = h * 4 + q
                        nc.tensor.transpose(
                            ptr[:, 64 * k : 64 * (k + 1)],
                            lgs[64 * h : 64 * (h + 1), 128 * q : 128 * (q + 1)],
                            i64[64 * h : 64 * (h + 1), :],
                        )
                probs_sb = outp.tile([128, 512], f32, tag="probs")
                idx_sb = outp.tile([128, 16], mybir.dt.uint32, tag="idx")
                wts_sb = outp.tile([128, 16], f32, tag="wts")
                for k in range(8):
                    pt = ptr[:, 64 * k : 64 * (k + 1)]
                    pslice = probs_sb[:, 64 * k : 64 * (k + 1)]
                    mneg = smp.tile([128, 1], f32, tag="mneg")
                    nc.vector.tensor_reduce(
                        mneg[:], pt, axis=mybir.AxisListType.X,
                        op=mybir.AluOpType.max, negate=True,
                    )
                    ssum = smp.tile([128, 1], f32, tag="ssum")
                    nc.scalar.activation(
                        pslice, pt, mybir.ActivationFunctionType.Exp,
                        bias=mneg[:], scale=1.0, accum_out=ssum[:],
                    )
                    rr = smp.tile([128, 1], f32, tag="rr")
                    nc.vector.reciprocal(rr[:], ssum[:])
                    nc.scalar.activation(
                        pslice, pslice, mybir.ActivationFunctionType.Copy,
                        scale=rr[:],
                    )
                    mx = smp.tile([128, 8], f32, tag="mx")
                    nc.vector.max(mx[:], pslice)
                    ix = smp.tile([128, 8], mybir.dt.uint32, tag="ix")
                    nc.vector.max_index(ix[:], mx[:], pslice)
                    nc.vector.tensor_copy(idx_sb[:, 2 * k : 2 * k + 2], ix[:, 0:2])
                    s2 = smp.tile([128, 1], f32, tag="s2")
                    nc.vector.tensor_reduce(
                        s2[:], mx[:, 0:2], axis=mybir.AxisListType.X,
                        op=mybir.AluOpType.add,
                    )
                    r2 = smp.tile([128, 1], f32, tag="r2")
                    nc.vector.reciprocal(r2[:], s2[:])
                    nc.vector.tensor_scalar_mul(
                        wts_sb[:, 2 * k : 2 * k + 2], mx[:, 0:2], r2[:]
                    )
                # outputs for block b
                nc.scalar.dma_start(
                    probs_d[b * TB : (b + 1) * TB, :].rearrange(
                        "(k p) e -> p k e", p=128
                    ),
                    probs_sb[:],
                )
                nc.scalar.dma_start(
                    idx_d[b * TB : (b + 1) * TB, :].rearrange(
                        "(k p) e -> p k e", p=128
                    ),
                    idx_sb[:].bitcast(mybir.dt.int32),
                )
                nc.scalar.dma_start(
                    wts_d[b * TB : (b + 1) * TB, :].rearrange(
                        "(k p) e -> p k e", p=128
                    ),
                    wts_sb[:],
                )

    _split_waits(nc, max_waits=1)
    return nc


_NC_CACHE = {}


def _get_nc():
    if "nc" not in _NC_CACHE:
        _NC_CACHE["nc"] = build_nc()
    return _NC_CACHE["nc"]


def kernel(x, W):
    from concourse.bass_utils import run_bass_kernel_spmd

    x = np.asarray(x, dtype=np.float32)
    W = np.asarray(W, dtype=np.float32)
    B, S, _ = x.shape  # (4, 8192, 4096)
    xt = np.ascontiguousarray(x.reshape(B * S, D))
    i128 = np.eye(128, dtype=np.float32)
    i64 = np.concatenate([np.eye(64, dtype=np.float32)] * 2, axis=0)
    in_maps = [
        {
            "x": np.ascontiguousarray(xt[T_CORE * i : T_CORE * (i + 1)]),
            "w": W,
            "i128": i128,
            "i64": i64,
        }
        for i in range(N_CORES)
    ]
    nc = _get_nc()
    r = run_bass_kernel_spmd(nc, in_maps, core_ids=list(range(N_CORES)))
    probs = np.concatenate([r.results[i]["probs"] for i in range(N_CORES)], axis=0)
    idx = np.concatenate([r.results[i]["idx"] for i in range(N_CORES)], axis=0)
    wts = np.concatenate([r.results[i]["wts"] for i in range(N_CORES)], axis=0)
    return (
        probs.reshape(B, S, E),
        idx.reshape(B, S, 2).astype(np.int32),
        wts.reshape(B, S, 2),
    )


# revision 19
# speedup vs baseline: 1.2028x; 1.0070x over previous
"""MoE router kernel for Trainium2 (8 NeuronCores, SPMD data-parallel).

logits = x @ W.T ; probs = softmax(logits) ; top-2 + renormalized weights.

Sharding: x (4,8192,4096) -> (32768,4096) tokens, 4096 tokens per core;
W (64,4096) replicated.

Per-core pipeline (all fp32):
  1. DMA x block natural [128t, 1024d] tiles.
  2. PE transpose-mode flips 128x128 tiles -> PSUM (x.T layout).
  3. DVE/ACT drain PSUM -> SBUF [128d, 512t] fp32 tiles.
  4. PE matmuls, col-tiled 2x (two 512-token halves concurrently in
     col-groups 0-1 / 2-3): psum[0:64]=W.T_c.T @ xT_A, psum[64:128]=...B,
     accumulated over 32 d-chunks -> logits.T [2x64e, 512t].
  5. PE transposes logits back to [128t, 64e]; softmax on DVE/ACT
     (exp with fused row-sum accum), top-2 via DVE max/max_index.
  6. DMA probs/indices/weights out.
"""
import sys

sys.path.insert(0, "/opt/trn_rl_repo")

import numpy as np

N_CORES = 8
T_CORE = 4096  # tokens per core
D = 4096
E = 64
TB = 1024  # token block
DS = 1024  # d slice
N_BLOCKS = T_CORE // TB
N_SLICES = D // DS


def _split_waits(nc, max_waits=1):
    """walrus rejects instructions with >N sem waits; push overflow waits
    onto NOPs inserted before, on the same engine."""
    from concourse import mybir

    for f in nc.m.functions:
        for blk in f.blocks:
            insts = list(blk.instructions)
            out = []
            changed = False
            for inst in insts:
                si = inst.sync_info
                waits = list(si.on_wait) if si is not None else []
                if len(waits) > max_waits:
                    overflow = waits[:-max_waits]
                    keep = waits[-max_waits:]
                    for i in range(0, len(overflow), max_waits):
                        chunk = overflow[i : i + max_waits]
                        nop = mybir.InstNoOp(
                            name=nc.get_next_instruction_name(),
                            engine=inst.engine,
                            ins=[],
                            outs=[],
                            sync_info=mybir.SyncInfo(on_wait=chunk, on_update=[]),
                        )
                        nc.register_instruction(nop)
                        out.append(nop)
                    inst.sync_info = mybir.SyncInfo(
                        on_wait=keep, on_update=list(si.on_update)
                    )
                    changed = True
                out.append(inst)
            if changed:
                blk.instructions = out


def build_nc():
    import concourse.bass as bass
    import concourse.mybir as mybir
    from concourse import tile

    f32 = mybir.dt.float32

    nc = bass.Bass()
    x_d = nc.dram_tensor("x", [T_CORE, D], f32, kind="ExternalInput")
    w_d = nc.dram_tensor("w", [E, D], f32, kind="ExternalInput")
    i128_d = nc.dram_tensor("i128", [128, 128], f32, kind="ExternalInput")
    i64_d = nc.dram_tensor("i64", [128, 64], f32, kind="ExternalInput")
    probs_d = nc.dram_tensor("probs", [T_CORE, E], f32, kind="ExternalOutput")
    idx_d = nc.dram_tensor("idx", [T_CORE, 2], mybir.dt.int32, kind="ExternalOutput")
    wts_d = nc.dram_tensor("wts", [T_CORE, 2], f32, kind="ExternalOutput")

    with tile.TileContext(nc) as tc:
        with (
            tc.tile_pool(name="const", bufs=1) as constp,
            tc.tile_pool(name="xn", bufs=24) as xnp,
            tc.tile_pool(name="xt", bufs=6) as xtp,
            tc.tile_pool(name="sm", bufs=8) as smp,
            tc.tile_pool(name="outp", bufs=2) as outp,
            tc.tile_pool(name="pslg", bufs=1, space="PSUM") as pslg,
            tc.tile_pool(name="pslg2", bufs=1, space="PSUM") as pslg2,
            tc.tile_pool(name="pstr", bufs=3, space="PSUM") as pstr,
            tc.tile_pool(name="psptr", bufs=1, space="PSUM") as psptr,
        ):
            def load_slice(b, s_):
                tiles = []
                for j in range(8):
                    xn = xnp.tile([128, DS], f32, tag="xn")
                    nc.sync.dma_start(
                        xn[:],
                        x_d[
                            b * TB + j * 128 : b * TB + (j + 1) * 128,
                            s_ * DS : (s_ + 1) * DS,
                        ],
                    )
                    tiles.append(xn)
                return tiles

            i128 = constp.tile([128, 128], f32)
            nc.sync.dma_start(i128[:], i128_d[:, :])
            i64 = constp.tile([128, 64], f32)
            nc.sync.dma_start(i64[:], i64_d[:, :])
            w_sb = constp.tile([64, D], f32)
            nc.sync.dma_start(w_sb[:], w_d[:, :])

            first_tiles = load_slice(0, 0)

            # ---- W.T setup: wt[:, 64c:64c+64] = W[:, 128c:128c+128].T ----
            wt = constp.tile([128, 32 * E], f32)  # 32 chunks of [128d, 64e]
            for g in range(4):
                pw = pstr.tile([128, 512], f32, tag="tr")
                for q in range(8):
                    c = g * 8 + q
                    nc.tensor.transpose(
                        pw[:, 64 * q : 64 * (q + 1)],
                        w_sb[:, 128 * c : 128 * (c + 1)],
                        i64[0:64, :],
                    )
                nc.vector.tensor_copy(wt[:, 512 * g : 512 * (g + 1)], pw[:])

            # ---- main loop ----
            for b in range(N_BLOCKS):
                lgA = pslg.tile([128, 512], f32, tag="lgA")
                lgB = pslg2.tile([128, 512], f32, tag="lgB")
                for s in range(N_SLICES):
                    if b == 0 and s == 0:
                        xts = first_tiles
                    else:
                        xts = load_slice(b, s)
                    for c in range(8):
                        kchunk = s * 8 + c
                        first = kchunk == 0
                        last = kchunk == 31
                        xt_sb = xtp.tile([128, 1024], f32, tag="xt")
                        for h in range(2):
                            tr = pstr.tile([128, 512], f32, tag="tr")
                            for q in range(4):
                                nc.tensor.transpose(
                                    tr[:, 128 * q : 128 * (q + 1)],
                                    xts[h * 4 + q][:, 128 * c : 128 * (c + 1)],
                                    i128[:],
                                )
                            if h == 0:
                                nc.vector.tensor_copy(xt_sb[:, 0:512], tr[:])
                            else:
                                nc.scalar.copy(xt_sb[:, 512:1024], tr[:])
                        xT = [xt_sb[:, 0:512], xt_sb[:, 512:1024]]
                        wchunk = wt[:, 64 * kchunk : 64 * (kchunk + 1)]
                        boost = 32 if (b > 0 or kchunk >= 2) else 0
                        with tc.high_priority(offset=boost):
                            nc.tensor.matmul(
                                lgA[0:64, :], wchunk, xT[0],
                                start=first, stop=last, tile_position=(0, 0),
                            )
                            nc.tensor.matmul(
                                lgB[64:128, :], wchunk, xT[1],
                                start=first, stop=last, tile_position=(0, 64),
                            )

                # logits.T for block b: lg = [2 halves x 64e, 512t]
                lgs = outp.tile([128, 512], f32, tag="lgs")
                nc.vector.tensor_copy(lgs[0:64, :], lgA[0:64, :])
                nc.scalar.copy(lgs[64:128, :], lgB[64:128, :])
                ptr = psptr.tile([128, 512], f32, tag="ptr")
                for h in range(2):
                    for q in range(4):
                        k = h * 4 + q
                        nc.tensor.transpose(
                            ptr[:, 64 * k : 64 * (k + 1)],
                            lgs[64 * h : 64 * (h + 1), 128 * q : 128 * (q + 1)],
                            i64[64 * h : 64 * (h + 1), :],
                        )
                probs_sb = outp.tile([128, 512], f32, tag="probs")
                idx_sb = outp.tile([128, 16], mybir.dt.uint32, tag="idx")
                wts_sb = outp.tile([128, 16], f32, tag="wts")
                for k in range(8):
                    pt = ptr[:, 64 * k : 64 * (k + 1)]
                    pslice = probs_sb[:, 64 * k : 64 * (k + 1)]
                    mneg = smp.tile([128, 1], f32, tag="mneg")
                    nc.vector.tensor_reduce(
                        mneg[:], pt, axis=mybir.AxisListType.X,
                        op=mybir.AluOpType.max, negate=True,
                    )
                    ssum = smp.tile([128, 1], f32, tag="ssum")
                    nc.scalar.activation(
                        pslice, pt, mybir.ActivationFunctionType.Exp,
                        bias=mneg[:], scale=1.0, accum_out=ssum[:],
                    )
                    rr = smp.tile([128, 1], f32, tag="rr")
                    nc.vector.reciprocal(rr[:], ssum[:])
                    nc.scalar.activation(
                        pslice, pslice, mybir.ActivationFunctionType.Copy,
                        scale=rr[:],
                    )
                    mx = smp.tile([128, 8], f32, tag="mx")
                    nc.vector.max(mx[:], pslice)
                    ix = smp.tile([128, 8], mybir.dt.uint32, tag="ix")
                    nc.vector.max_index(ix[:], mx[:], pslice)
                    nc.vector.tensor_copy(idx_sb[:, 2 * k : 2 * k + 2], ix[:, 0:2])
                    s2 = smp.tile([128, 1], f32, tag="s2")
                    nc.vector.tensor_reduce(
                        s2[:], mx[:, 0:2], axis=mybir.AxisListType.X,
                        op=mybir.AluOpType.add,
                    )
                    r2 = smp.tile([128, 1], f32, tag="r2")
                    nc.vector.reciprocal(r2[:], s2[:])
                    nc.vector.tensor_scalar_mul(
                        wts_sb[:, 2 * k : 2 * k + 2], mx[:, 0:2], r2[:]
                    )
                # outputs for block b
                nc.scalar.dma_start(
                    probs_d[b * TB : (b + 1) * TB, :].rearrange(
                        "(k p) e -> p k e", p=128
                    ),
                    probs_sb[:],
                )
                nc.scalar.dma_start(
                    idx_d[b * TB : (b + 1) * TB, :].rearrange(
                        "(k p) e -> p k e", p=128
                    ),
                    idx_sb[:].bitcast(mybir.dt.int32),
                )
                nc.scalar.dma_start(
                    wts_d[b * TB : (b + 1) * TB, :].rearrange(
                        "(k p) e -> p k e", p=128
                    ),
                    wts_sb[:],
                )

    _split_waits(nc, max_waits=1)
    return nc


_NC_CACHE = {}


def _get_nc():
    if "nc" not in _NC_CACHE:
        _NC_CACHE["nc"] = build_nc()
    return _NC_CACHE["nc"]


def kernel(x, W):
    from concourse.bass_utils import run_bass_kernel_spmd

    x = np.asarray(x, dtype=np.float32)
    W = np.asarray(W, dtype=np.float32)
    B, S, _ = x.shape  # (4, 8192, 4096)
    xt = np.ascontiguousarray(x.reshape(B * S, D))
    i128 = np.eye(128, dtype=np.float32)
    i64 = np.concatenate([np.eye(64, dtype=np.float32)] * 2, axis=0)
    in_maps = [
        {
            "x": np.ascontiguousarray(xt[T_CORE * i : T_CORE * (i + 1)]),
            "w": W,
            "i128": i128,
            "i64": i64,
        }
        for i in range(N_CORES)
    ]
    nc = _get_nc()
    r = run_bass_kernel_spmd(nc, in_maps, core_ids=list(range(N_CORES)))
    probs = np.concatenate([r.results[i]["probs"] for i in range(N_CORES)], axis=0)
    idx = np.concatenate([r.results[i]["idx"] for i in range(N_CORES)], axis=0)
    wts = np.concatenate([r.results[i]["wts"] for i in range(N_CORES)], axis=0)
    return (
        probs.reshape(B, S, E),
        idx.reshape(B, S, 2).astype(np.int32),
        wts.reshape(B, S, 2),
    )
